# revision 15
# baseline (speedup 1.0000x reference)
"""AFT-Full attention kernel for 8 TRN2 NeuronCores.

Data-parallel over batch B=8 (one batch element per core). Per core:
  Q = x_q @ wq + wq_b          [2048, 256]
  K = x_kv @ wk + wk_b         [2048, 256]
  V = x_kv @ wv + wv_b         [2048, 256]
  num = exp(bias) @ (exp(K)*V) [2048, 256]
  den = exp(bias) @ exp(K)     [2048, 256]
  Yt  = sigmoid(Q) * num / den
  out = Yt @ f2_w + f2_b       [2048, 256]

Schedule (v2): the kernel is PE-stream-bound (~106 us of matmul columns)
inside a ~95 us saturated DMA window, so the schedule's job is to keep
the PE fed from the first byte to the last:

- Phase 1 interleaves chunk-0 of the bias path (transpose+exp+num/den,
  lagged two groups so each bias quarter has arrived) into the K/V group
  loop: C-chunk work is the only work dense enough to fill the DMA-bound
  K/V stretch, and its num/den accumulation only needs the ekv/expk
  groups already produced. The DMA queue delivers bias c0 quarters
  between x_kv groups to match.
- x_q chunk 0's transposes ride along with group 7; its Q matmuls and
  the chunk-0 epilogue run in the phase-1 tail on the freed K/V psum
  banks (by then the DMA stream is nearly drained, so the tail also
  keeps the PE busy where it would otherwise idle).
- Phase 2 runs chunks 1-3 with the B-phase (x_q transposes + Q DR mms)
  spread inside each chunk's tb loop, the previous chunk's f2 deferred
  to tb==3, and the sigmoid/num/den epilogue at chunk boundaries
  overlapping the next chunk's first transposes.
- The last chunk splits its final num/den matmuls and epilogue/f2/store
  by s-halves to shorten the end-of-kernel tail (which runs
  HAM-throttled at half PE clock).

Engine assignment: casting DMAs can only issue from gpsimd, so gpsimd
hosts the load descriptor-gen, emitted in per-group/per-chunk batches
~1.5 segments ahead of consumption and interleaved with gpsimd's only
compute (half of the expw8 fp8 casts, SBUF->SBUF since GPSIMD has no
PSUM port). Store descriptor-gen lives on DVE, directly after the
out-tile copies it depends on, so it never blocks. ACT is near-saturated
by the exp()s (the only exp-capable engine) and takes nothing else
heavy; all other PSUM reads stay on DVE.

Precision split (inherited from v1, hard-won): num is a SIGNED
accumulation, so every operand on the num path (x_kv, wk/wv, exp(K)*V,
exp(bias)) stays bf16. den is all-positive and Q only feeds a sigmoid
gate; both run fp8e4 MatmulPerfMode.DoubleRow (halved PE rows). The
output is stored bf16 (host casts back to f32).
"""

import os
import numpy as np
from contextlib import ExitStack

import concourse.bass as bass
import concourse.tile as tile
from concourse import bacc, mybir
from concourse.bass_utils import run_bass_kernel_spmd
from concourse.masks import make_identity

F32 = mybir.dt.float32
BF16 = mybir.dt.bfloat16
FP8 = mybir.dt.float8e4

S = 2048   # n_q
T = 2048   # n_kv
D = 1024   # d_q == d_kv
H = 256    # hidden
G = 256    # output dim
P = 128    # partitions
SCH = 512  # s-chunk for the C phase (one PSUM bank of fp32)
NSB = SCH // P       # 4 row-blocks per chunk
NCH = S // SCH       # 4 chunks
NT = T // P          # 16 t row-blocks
NG = NT // 2         # 8 groups of 2 t-blocks
ND = D // P          # 8 d tiles
NDD = ND // 2        # 4 d-tile pairs (DoubleRow)
NHB = H // P         # 2 h blocks
TQ = T // 4          # bias quarter length along t

DR = mybir.MatmulPerfMode.DoubleRow
AFT = mybir.ActivationFunctionType


def _build(use_wq_b, use_wk_b, use_wv_b, use_f2_b):
    """Build the per-core Bass graph. Returns the compiled Bacc."""
    nc = bacc.Bacc(
        "TRN2",
        target_bir_lowering=False,
        debug=False,
        enable_asserts=False,
        num_devices=8,
    )

    x_q = nc.declare_dram_parameter("x_q", [S, D], F32, isOutput=False)
    x_kv = nc.declare_dram_parameter("x_kv", [T, D], F32, isOutput=False)
    bias = nc.declare_dram_parameter("bias", [S, T], F32, isOutput=False)
    wq_w = nc.declare_dram_parameter("wq_w", [D, H], F32, isOutput=False)
    wk_w = nc.declare_dram_parameter("wk_w", [D, H], F32, isOutput=False)
    wv_w = nc.declare_dram_parameter("wv_w", [D, H], F32, isOutput=False)
    f2_w = nc.declare_dram_parameter("f2_w", [H, G], F32, isOutput=False)
    wq_b = nc.declare_dram_parameter("wq_b", [1, H], F32, isOutput=False) if use_wq_b else None
    wk_b = nc.declare_dram_parameter("wk_b", [1, H], F32, isOutput=False) if use_wk_b else None
    wv_b = nc.declare_dram_parameter("wv_b", [1, H], F32, isOutput=False) if use_wv_b else None
    f2_b = nc.declare_dram_parameter("f2_b", [1, G], F32, isOutput=False) if use_f2_b else None
    out = nc.declare_dram_parameter("out", [S, G], BF16, isOutput=True)

    # DRAM views with the chunk/block structure exposed.
    xq_v = x_q[:].rearrange("(c sb p) d -> c p sb d", p=P, sb=NSB)     # [NCH,P,NSB,D]
    xkv_v = x_kv[:].rearrange("(g j p) d -> g p j d", p=P, j=2)        # [NG,P,2,D]
    bias_v = bias[:].rearrange("(c sb p) (q t) -> c q p sb t", p=P, sb=NSB, q=4)
    out_v = out[:].rearrange("(c sb p) g -> c p sb g", p=P, sb=NSB)
    wk_v = wk_w[:].rearrange("(dt p) h -> p dt h", p=P)
    wv_v = wv_w[:].rearrange("(dt p) h -> p dt h", p=P)

    with tile.TileContext(nc) as tc, ExitStack() as ctx:
        consts = ctx.enter_context(tc.tile_pool(name="consts", bufs=1))

        ident = consts.tile([P, P], BF16)

        # Staging pools sized so the SWDGE queue never waits on a slot.
        xkv_nat = ctx.enter_context(tc.tile_pool(name="xkv_nat", bufs=2 * NG))
        xq_nat = ctx.enter_context(tc.tile_pool(name="xq_nat", bufs=NCH))
        bias_nat = ctx.enter_context(tc.tile_pool(name="bias_nat", bufs=12))
        xT_sb = ctx.enter_context(tc.tile_pool(name="xT_sb", bufs=8))
        xqT2_sb = ctx.enter_context(tc.tile_pool(name="xqT2_sb", bufs=8))
        expw_sb = ctx.enter_context(tc.tile_pool(name="expw_sb", bufs=6))
        expw8_sb = ctx.enter_context(tc.tile_pool(name="expw8_sb", bufs=4))
        epi = ctx.enter_context(tc.tile_pool(name="epi", bufs=3))
        out_sb_pool = ctx.enter_context(tc.tile_pool(name="out_sb", bufs=2))

        wkv_sb = consts.tile([P, ND, 2 * H], BF16, tag="w_wkv", name="wkv_sb")
        wq_bf = consts.tile([P, ND, H], BF16, tag="w_wq", name="wq_bf")
        wq_sb = consts.tile([P, ND, H], FP8, tag="w_wq8", name="wq_sb")
        f2_sb = consts.tile([P, NHB, G], BF16, tag="w_f2")

        # ---- load batches: emitted progressively on the gpsimd FIFO so
        # descriptor-gen stays ~1.5 segments ahead of consumption without a
        # long serial prefix (gpsimd also runs half the expw8 casts).
        xkv_tiles = {}
        bias_tiles = {}
        xq_tiles = {}

        def load_xkv(g):
            for j_ in range(2):
                t_ = xkv_nat.tile([P, D], BF16, tag="xkv_nat", name="xkv_nat")
                nc.gpsimd.dma_start(t_[:], xkv_v[g][:, j_, :])
                xkv_tiles[(g, j_)] = t_

        def load_bias_q(c, q):
            t_ = bias_nat.tile([P, NSB, TQ], BF16, tag="bias_nat", name="bias_nat")
            nc.gpsimd.dma_start(t_[:], bias_v[c][q])
            bias_tiles[(c, q)] = t_

        def load_xq(c):
            t_ = xq_nat.tile([P, NSB, D], BF16, tag="xq_nat", name="xq_nat")
            nc.gpsimd.dma_start(t_[:], xq_v[c])
            xq_tiles[c] = t_

        def load_wkv(half):
            dsl = slice(half * 4, half * 4 + 4)
            nc.gpsimd.dma_start(wkv_sb[:, dsl, 0:H], wk_v[:, dsl, :])
            nc.gpsimd.dma_start(wkv_sb[:, dsl, H : 2 * H], wv_v[:, dsl, :])

        def load_wq():
            nc.gpsimd.dma_start(wq_bf[:], wq_w[:].rearrange("(dt p) h -> p dt h", p=P))

        def load_f2():
            nc.gpsimd.dma_start(f2_sb[:], f2_w[:].rearrange("(ht p) g -> p ht g", p=P))

        # Upfront: just enough for group 0 + the first c0 quarter.
        load_xkv(0)
        make_identity(nc, ident[:])
        load_wkv(0)
        load_wkv(1)
        load_xkv(1)
        load_bias_q(0, 0)

        # Remaining loads, all upfront in consumption order: the gpsimd FIFO
        # carries ONLY descriptor-gen (and phase-2 Pool casts emitted after
        # every load), so the DMA stream can never be blocked by compute.
        load_xkv(2)
        load_xkv(3)
        load_bias_q(0, 1)
        load_xkv(4)
        load_xkv(5)
        load_bias_q(0, 2)
        load_wq()
        load_xkv(6)
        load_xq(0)
        load_xkv(7)
        load_bias_q(0, 3)
        load_bias_q(1, 0)
        load_xq(1)
        load_bias_q(1, 1)
        load_f2()
        load_bias_q(1, 2)
        load_bias_q(1, 3)
        load_xq(2)
        load_bias_q(2, 0)
        load_bias_q(2, 1)
        load_bias_q(2, 2)
        load_xq(3)
        load_bias_q(2, 3)
        load_bias_q(3, 0)
        load_bias_q(3, 1)
        load_bias_q(3, 2)
        load_bias_q(3, 3)

        bias_vecs = {}
        ones_row = None
        if any(b is not None for b in (wq_b, wk_b, wv_b, f2_b)):
            ones_row = consts.tile([1, SCH], BF16)
            nc.gpsimd.memset(ones_row[:], 1.0)
            for name, b in (("wq", wq_b), ("f2", f2_b)):
                if b is not None:
                    bt = consts.tile([1, H], BF16, tag=f"b_{name}", name="bt")
                    nc.gpsimd.dma_start(bt[:], b[:])
                    bias_vecs[name] = bt
            if wk_b is not None or wv_b is not None:
                bkv = consts.tile([1, 2 * H], BF16, tag="b_kv", name="bkv")
                nc.gpsimd.memset(bkv[:], 0.0)
                if wk_b is not None:
                    nc.gpsimd.dma_start(bkv[:, 0:H], wk_b[:])
                if wv_b is not None:
                    nc.gpsimd.dma_start(bkv[:, H : 2 * H], wv_b[:])
                bias_vecs["kv"] = bkv

        # Long-lived activations.
        enq_pool = ctx.enter_context(tc.tile_pool(name="enq", bufs=NHB))
        enq = [enq_pool.tile([P, S], BF16, tag="enq", name="enq") for _ in range(NHB)]
        ek_pool = ctx.enter_context(tc.tile_pool(name="expk", bufs=NG))
        ek8_pool = ctx.enter_context(tc.tile_pool(name="expk8", bufs=NG))
        ekv_pool = ctx.enter_context(tc.tile_pool(name="ekv", bufs=NG))
        expk = [ek_pool.tile([P, 2 * H], BF16, tag="expk", name="expk") for _ in range(NG)]
        expk8 = [ek8_pool.tile([P, 2, H], FP8, tag="expk8", name="expk8") for _ in range(NG)]
        ekv = [ekv_pool.tile([P, 2 * H], BF16, tag="ekv", name="ekv") for _ in range(NG)]

        # num/den accumulators span phase 1 (chunk 0) and phase 2: own pool.
        psum_nd = ctx.enter_context(tc.tile_pool(name="psum_nd", bufs=4, space="PSUM"))

        # ---------------- shared helpers ----------------
        def kv_exps(g, ps_kv):
            for j in range(2):
                nc.scalar.activation(
                    expk[g][:, j * H : (j + 1) * H], ps_kv[j][:, 0:H], AFT.Exp
                )
                nc.vector.tensor_mul(
                    ekv[g][:, j * H : (j + 1) * H],
                    expk[g][:, j * H : (j + 1) * H],
                    ps_kv[j][:, H : 2 * H],
                )
                # fp8 copy on DVE (SBUF->SBUF) instead of a second ACT exp:
                # ACT drains the K/V psum ring faster and keeps its margin.
                nc.vector.tensor_copy(expk8[g][:, j, :], expk[g][:, j * H : (j + 1) * H])

        # C-phase state for the current chunk.
        cst = {}

        def c_open(c):
            cst["c"] = c
            cst["num"] = [psum_nd.tile([P, SCH], F32, tag="nd", name="ps_num") for _ in range(NHB)]
            cst["den"] = [psum_nd.tile([P, SCH], F32, tag="nd", name="ps_den") for _ in range(NHB)]
            cst["expw"] = {}
            cst["expw8"] = {}

        def c_tr(tb, tr_pool, tr_tag="tr"):
            c = cst["c"]
            nath = bias_tiles[(c, tb // 4)]
            tloc = (tb % 4) * P
            ps = tr_pool.tile([P, SCH], F32, tag=tr_tag, name="ps_trc")
            for sb in range(NSB):
                nc.tensor.matmul(
                    ps[:, sb * P : (sb + 1) * P],
                    nath[:, sb, tloc : tloc + P],
                    ident[:],
                )
            sbuf = expw_sb.tile([P, SCH], BF16, tag="expw", name="expw")
            nc.scalar.activation(sbuf[:], ps[:], AFT.Exp)
            cst["expw"][tb] = sbuf
            # fp8 copy for the den DoubleRow matmul. SBUF->SBUF, so mid-
            # chunk phase-2 copies can run on Pool (idle once desc-gen is
            # done; ~3x slower than DVE, so only where its latency hides);
            # the rest on DVE.
            g = tb // 2
            if tb % 2 == 0:
                cst["expw8"][g] = expw8_sb.tile([P, 2, SCH], FP8, tag="expw8", name="expw8")
            eng = nc.gpsimd if (cst["c"] > 0 and tb % 2 == 1 and tb <= 7) else nc.vector
            eng.tensor_copy(cst["expw8"][g][:, tb % 2, :], sbuf[:])

        def c_num(tb, first=None, last=None):
            first = (tb == 0) if first is None else first
            last = (tb == NT - 1) if last is None else last
            rhs = cst["expw"][tb][:]
            for hb in range(NHB):
                lo = (tb % 2) * H + hb * P
                nc.tensor.matmul(
                    cst["num"][hb][:],
                    ekv[tb // 2][:, lo : lo + P],
                    rhs,
                    start=first,
                    stop=last,
                )

        def c_num_half(tb, h):
            rhs = cst["expw"][tb][:, h * 2 * P : (h + 1) * 2 * P]
            for hb in range(NHB):
                lo = (tb % 2) * H + hb * P
                nc.tensor.matmul(
                    cst["num"][hb][:, h * 2 * P : (h + 1) * 2 * P],
                    ekv[tb // 2][:, lo : lo + P],
                    rhs,
                    start=False,
                    stop=True,
                )

        def c_den(g, first=None, last=None):
            first = (g == 0) if first is None else first
            last = (g == NG - 1) if last is None else last
            for hb in range(NHB):
                nc.tensor.matmul(
                    cst["den"][hb][:],
                    expk8[g][:, :, hb * P : (hb + 1) * P],
                    cst["expw8"][g][:],
                    start=first,
                    stop=last,
                    perf_mode=DR,
                )

        def c_den_half(g, h):
            for hb in range(NHB):
                nc.tensor.matmul(
                    cst["den"][hb][:, h * 2 * P : (h + 1) * 2 * P],
                    expk8[g][:, :, hb * P : (hb + 1) * P],
                    cst["expw8"][g][:, :, h * 2 * P : (h + 1) * 2 * P],
                    start=False,
                    stop=True,
                    perf_mode=DR,
                )

        # B-phase helpers (x_q transpose + Q DR matmul for chunk c).
        bst = {}

        def b_open(c):
            bst["c"] = c
            bst["xqT2"] = {}

        def b_tr(d, tr_pool, tr_tag="tr"):
            nat = xq_tiles[bst["c"]]
            ps = tr_pool.tile([P, SCH], F32, tag=tr_tag, name="ps_trq")
            for sb in range(NSB):
                nc.tensor.matmul(
                    ps[:, sb * P : (sb + 1) * P],
                    nat[:, sb, d * P : (d + 1) * P],
                    ident[:],
                )
            if d % 2 == 0:
                bst["xqT2"][d // 2] = xqT2_sb.tile(
                    [P, 2, SCH], FP8, tag="xqT2", name="xqT2"
                )
            if d < 2:
                nc.scalar.copy(bst["xqT2"][d // 2][:, d % 2, :], ps[:])
            else:
                nc.vector.tensor_copy(bst["xqT2"][d // 2][:, d % 2, :], ps[:])

        def b_mm(dd, q_pool, q_tag):
            if dd == 0:
                bst["ps_q"] = [
                    q_pool.tile([P, SCH], F32, tag=q_tag, name="ps_q") for _ in range(NHB)
                ]
            for hb in range(NHB):
                nc.tensor.matmul(
                    bst["ps_q"][hb][:],
                    wq_sb[:, 2 * dd : 2 * dd + 2, hb * P : (hb + 1) * P],
                    bst["xqT2"][dd][:],
                    start=(dd == 0),
                    stop=(dd == NDD - 1 and "wq" not in bias_vecs),
                    perf_mode=DR,
                )

        def b_close():
            c = bst["c"]
            for hb in range(NHB):
                if "wq" in bias_vecs:
                    nc.tensor.matmul(
                        bst["ps_q"][hb][:],
                        bias_vecs["wq"][:, hb * P : (hb + 1) * P],
                        ones_row[:],
                        start=False,
                        stop=True,
                    )
                nc.scalar.activation(
                    enq[hb][:, c * SCH : (c + 1) * SCH],
                    bst["ps_q"][hb][:],
                    AFT.Sigmoid,
                )

        def epilogue(c):
            # Yt^T = num^T * (recip(den^T) * sig(Q)^T). The recip/mul chain
            # depends only on den and the sigmoid gate, so when the dens
            # finish before the last num matmuls, only the final muls trail
            # the num-stop.
            rec = []
            for hb in range(NHB):
                r = epi.tile([P, SCH], F32, tag="rec", name="rec", bufs=2)
                nc.vector.reciprocal_approx_fast(r[:], cst["den"][hb][:])
                rec.append(r)
            m = []
            for hb in range(NHB):
                mm_ = epi.tile([P, SCH], F32, tag="d2", name="m", bufs=2)
                nc.vector.tensor_mul(
                    mm_[:], rec[hb][:], enq[hb][:, c * SCH : (c + 1) * SCH]
                )
                m.append(mm_)
            ytT = []
            for hb in range(NHB):
                yt = epi.tile([P, SCH], BF16, tag="yt", name="yt", bufs=4)
                nc.vector.tensor_mul(yt[:], m[hb][:], cst["num"][hb][:])
                ytT.append(yt)
            return ytT

        def emit_f2(ytT, c, f_pool, f_tag):
            # Deferred f2 projection + store for chunks 0-2.
            out_sb = out_sb_pool.tile([P, NSB, G], BF16, tag="out_sb", name="out_sb")
            for sb in range(NSB):
                ps_f = f_pool.tile([P, SCH], F32, tag=f_tag, name="ps_f")
                for hb in range(NHB):
                    nc.tensor.matmul(
                        ps_f[:, 0:G],
                        ytT[hb][:, sb * P : (sb + 1) * P],
                        f2_sb[:, hb, :],
                        start=(hb == 0),
                        stop=(hb == NHB - 1 and "f2" not in bias_vecs),
                    )
                if "f2" in bias_vecs:
                    nc.tensor.matmul(
                        ps_f[:, 0:G],
                        ones_row[:, 0:P],
                        bias_vecs["f2"][:],
                        start=False,
                        stop=True,
                    )
                nc.vector.tensor_copy(out_sb[:, sb, :], ps_f[:, 0:G])
            # Stores are non-casting -> hardware DGE on SP (own queue).
            nc.sync.dma_start(out_v[c], out_sb[:])

        # ---------------- Phase 1: A + C0 (+ B0 transposes) ----------------
        # One psum layout for the whole kernel (no pool close/open barrier at
        # the phase transition): trc holds every transpose, fq holds the K/V
        # accumulators, B ps_q, and f2 blocks -- all [P,512] tiles whose
        # lifetimes never overlap across uses.
        psum_trc = ctx.enter_context(tc.tile_pool(name="psum_trc", bufs=2, space="PSUM"))
        psum_fq = ctx.enter_context(tc.tile_pool(name="psum_fq", bufs=2, space="PSUM"))

        c_open(0)
        b_open(0)

        # Warm-up filler: ~4us of ident matmuls into a throwaway psum tile.
        # The PE would idle here waiting for the first x_kv tiles; staying
        # busy keeps the HAM clock gate from parking the PE at half speed
        # for the first real groups. The tile has no readers, so the ring
        # slot recycles on WAR against the PE itself.
        warm = psum_trc.tile([P, SCH], F32, tag="trc", name="ps_warm")
        for _ in range(20):
            for k4 in range(SCH // P):
                nc.tensor.matmul(warm[:, k4 * P : (k4 + 1) * P], ident[:], ident[:])

        for g in range(NG):
            ps_kv = [psum_fq.tile([P, 2 * H], F32, tag="fq", name="ps_kv") for _ in range(2)]
            xkvT = {}

            def a_tr2(dp, g=g, xkvT=xkvT):
                # Transpose d-pair (2d, 2d+1) x (j=0,1) into one psum bank;
                # a single [P,512] DVE cast amortizes psum-access + semaphore
                # overhead (DVE is phase-1's co-bottleneck).
                ps = psum_trc.tile([P, SCH], F32, tag="trc", name="ps_tr")
                for k in range(2):
                    for j in range(2):
                        nc.tensor.matmul(
                            ps[:, (2 * k + j) * P : (2 * k + j + 1) * P],
                            xkv_tiles[(g, j)][:, (2 * dp + k) * P : (2 * dp + k + 1) * P],
                            ident[:],
                        )
                t_ = xT_sb.tile([P, 4 * P], BF16, tag="xkvT", name="xkvT")
                nc.vector.tensor_copy(t_[:], ps[:])
                xkvT[dp] = t_

            def a_mm(d, ps_kv=ps_kv, xkvT=xkvT):
                t_ = xkvT[d // 2]
                for j in range(2):
                    nc.tensor.matmul(
                        ps_kv[j][:],
                        t_[:, ((d % 2) * 2 + j) * P : ((d % 2) * 2 + j + 1) * P],
                        wkv_sb[:, d, :],
                        start=(d == 0),
                        stop=(d == ND - 1 and "kv" not in bias_vecs),
                    )

            for d in range(ND):
                if d % 2 == 0:
                    a_tr2(d // 2)
                # Lagged chunk-0 C work: group g handles tb = 2(g-2),
                # 2(g-2)+1 transposes, tb = 2(g-3)(+1) num, and den(g-3).
                if d == 1 and g >= 2:
                    c_tr(2 * (g - 2), psum_trc, "trc")
                if d == 2 and g >= 3:
                    c_num(2 * (g - 3))
                if d >= 3:
                    a_mm(d - 3)
                if d == 4 and g >= 2:
                    c_tr(2 * (g - 2) + 1, psum_trc, "trc")
                if d == 5 and g >= 3:
                    c_num(2 * (g - 3) + 1)
                if d == 6 and g >= 3:
                    c_den(g - 3)
                # B0 transposes ride along with group 7.
                if g == 7 and d % 2 == 1:
                    b_tr((d - 1) // 2, psum_trc, "trc")
            a_mm(ND - 3)
            a_mm(ND - 2)
            a_mm(ND - 1)
            if "kv" in bias_vecs:
                for j in range(2):
                    nc.tensor.matmul(
                        ps_kv[j][:],
                        ones_row[:, 0:P],
                        bias_vecs["kv"][:],
                        start=False,
                        stop=True,
                    )
            kv_exps(g, ps_kv)
            # wq bf16 -> fp8 conversion on ACT, spread over groups 5-7 (wq
            # lands mid-phase-1; each piece is one DR stationary pair).
            if g >= 5:
                dd = g - 5
                nc.scalar.copy(
                    wq_sb[:, 2 * dd : 2 * dd + 2, :], wq_bf[:, 2 * dd : 2 * dd + 2, :]
                )

        # Phase-1 tail: finish chunk 0 and B0 on the freed K/V banks. By now
        # the load queue is nearly drained, so this also keeps the PE busy.
        c_tr(12, psum_trc, "trc")
        c_num(10)
        b_tr(4, psum_trc, "trc")
        c_tr(13, psum_trc, "trc")
        c_num(11)
        c_den(5)
        nc.scalar.copy(wq_sb[:, 6:8, :], wq_bf[:, 6:8, :])
        b_tr(5, psum_trc, "trc")
        c_tr(14, psum_trc, "trc")
        c_num(12)
        b_tr(6, psum_trc, "trc")
        c_tr(15, psum_trc, "trc")
        c_num(13)
        b_tr(7, psum_trc, "trc")
        c_den(6)
        b_mm(0, psum_fq, "fq")
        c_den(7)
        b_mm(1, psum_fq, "fq")
        c_num(14)
        b_mm(2, psum_fq, "fq")
        c_num(15)
        b_mm(3, psum_fq, "fq")
        b_close()
        ytT0 = epilogue(0)
        pending_f2 = [(ytT0, 0)]

        # ---------------- Phase 2: chunks 1-3 ----------------
        if True:
            for c in range(1, NCH):
                c_open(c)
                b_open(c)
                last = c == NCH - 1
                # tb visit order: 10,11 go last so every den group (and
                # den's DVE casts) completes before the final num matmuls --
                # the epilogue's recip/mul chain then overlaps nums 10/11.
                seq = list(range(10)) + [12, 13, 14, 15, 10, 11]
                den_at = {4: 0, 6: 1, 8: 2, 10: 3, 12: 4, 13: 6, 15: 7}
                for pos in range(NT):
                    tb = seq[pos]
                    c_tr(tb, psum_trc, "trc")
                    if pos == 5 and pending_f2[0] is not None:
                        emit_f2(*pending_f2[0], psum_fq, "fq")
                        pending_f2[0] = None
                    if pos >= 2:
                        tn = seq[pos - 2]
                        c_num(tn, first=(tn == 0), last=False)
                    if 3 <= pos <= 10:
                        b_tr(pos - 3, psum_trc, "trc")
                    if pos in (7, 9, 11, 13):
                        b_mm((pos - 7) // 2, psum_fq, "fq")
                    if pos in den_at:
                        g_ = den_at[pos]
                        c_den(g_, first=(g_ == 0), last=False)
                    if pos == 14:
                        b_close()
                # dens finish (den 5 last), then the trailing nums.
                c_den(5, first=False, last=True)
                c_num(10, first=False, last=False)
                c_num(11, first=False, last=True)
                ytT = epilogue(c)
                if not last:
                    pending_f2[0] = (ytT, c)
                else:
                    out_sb = out_sb_pool.tile([P, NSB, G], BF16, tag="out_sb", name="out_sb")
                    for sb in range(NSB):
                        ps_f = psum_fq.tile([P, SCH], F32, tag="fq", name="ps_fh")
                        for hb in range(NHB):
                            nc.tensor.matmul(
                                ps_f[:, 0:G],
                                ytT[hb][:, sb * P : (sb + 1) * P],
                                f2_sb[:, hb, :],
                                start=(hb == 0),
                                stop=(hb == NHB - 1 and "f2" not in bias_vecs),
                            )
                        if "f2" in bias_vecs:
                            nc.tensor.matmul(
                                ps_f[:, 0:G],
                                ones_row[:, 0:P],
                                bias_vecs["f2"][:],
                                start=False,
                                stop=True,
                            )
                        nc.vector.tensor_copy(out_sb[:, sb, :], ps_f[:, 0:G])
                        nc.sync.dma_start(out_v[c][:, sb, :], out_sb[:, sb, :])

    nc.compile()
    return nc


_CACHE = {}


def _get_nc(use_wq_b, use_wk_b, use_wv_b, use_f2_b):
    key = (use_wq_b, use_wk_b, use_wv_b, use_f2_b)
    if key not in _CACHE:
        _CACHE[key] = _build(*key)
    return _CACHE[key]


def kernel(x_q, x_kv, bias, wq_w, wq_b, wk_w, wk_b, wv_w, wv_b, f2_w, f2_b,
           _trace=False, _trace_kwargs=None):
    x_q = np.ascontiguousarray(np.asarray(x_q, dtype=np.float32))
    x_kv = np.ascontiguousarray(np.asarray(x_kv, dtype=np.float32))
    bias = np.ascontiguousarray(np.asarray(bias, dtype=np.float32))
    wq_w = np.ascontiguousarray(np.asarray(wq_w, dtype=np.float32))
    wk_w = np.ascontiguousarray(np.asarray(wk_w, dtype=np.float32))
    wv_w = np.ascontiguousarray(np.asarray(wv_w, dtype=np.float32))
    f2_w = np.ascontiguousarray(np.asarray(f2_w, dtype=np.float32))
    wq_b = np.asarray(wq_b, dtype=np.float32)
    wk_b = np.asarray(wk_b, dtype=np.float32)
    wv_b = np.asarray(wv_b, dtype=np.float32)
    f2_b = np.asarray(f2_b, dtype=np.float32)

    use_b = tuple(bool(np.any(b)) for b in (wq_b, wk_b, wv_b, f2_b))
    nc = _get_nc(*use_b)

    n_cores = 8
    in_maps = []
    for i in range(n_cores):
        m = {
            "x_q": x_q[i],
            "x_kv": x_kv[i],
            "bias": bias[i],
            "wq_w": wq_w,
            "wk_w": wk_w,
            "wv_w": wv_w,
            "f2_w": f2_w,
        }
        if use_b[0]:
            m["wq_b"] = wq_b.reshape(1, H)
        if use_b[1]:
            m["wk_b"] = wk_b.reshape(1, H)
        if use_b[2]:
            m["wv_b"] = wv_b.reshape(1, H)
        if use_b[3]:
            m["f2_b"] = f2_b.reshape(1, G)
        in_maps.append(m)

    if not _trace:
        # The NTFF trace hook is unavailable outside the dev harness; make
        # sure a stray BASS_TRACE env var cannot route us onto that path.
        os.environ["BASS_NEVER_TRACE"] = "1"
    else:
        os.environ.pop("BASS_NEVER_TRACE", None)
    res = run_bass_kernel_spmd(
        nc, in_maps, list(range(n_cores)), trace=_trace, **(_trace_kwargs or {})
    )
    out = np.stack(
        [np.asarray(res.results[i]["out"]).astype(np.float32) for i in range(n_cores)],
        axis=0,
    )
    if _trace:
        return out, res
    return out


# revision 16
# speedup vs baseline: 1.0382x; 1.0382x over previous
"""AFT-Full attention kernel for 8 TRN2 NeuronCores.

Data-parallel over batch B=8 (one batch element per core). Per core:
  Q = x_q @ wq + wq_b          [2048, 256]
  K = x_kv @ wk + wk_b         [2048, 256]
  V = x_kv @ wv + wv_b         [2048, 256]
  num = exp(bias) @ (exp(K)*V) [2048, 256]
  den = exp(bias) @ exp(K)     [2048, 256]
  Yt  = sigmoid(Q) * num / den
  out = Yt @ f2_w + f2_b       [2048, 256]

Schedule (v2): the kernel is PE-stream-bound (~106 us of matmul columns)
inside a ~95 us saturated DMA window, so the schedule's job is to keep
the PE fed from the first byte to the last:

- Phase 1 interleaves chunk-0 of the bias path (transpose+exp+num/den,
  lagged two groups so each bias quarter has arrived) into the K/V group
  loop: C-chunk work is the only work dense enough to fill the DMA-bound
  K/V stretch, and its num/den accumulation only needs the ekv/expk
  groups already produced. The DMA queue delivers bias c0 quarters
  between x_kv groups to match.
- x_q chunk 0's transposes ride along with group 7; its Q matmuls and
  the chunk-0 epilogue run in the phase-1 tail on the freed K/V psum
  banks (by then the DMA stream is nearly drained, so the tail also
  keeps the PE busy where it would otherwise idle).
- Phase 2 runs chunks 1-3 with the B-phase (x_q transposes + Q DR mms)
  spread inside each chunk's tb loop, the previous chunk's f2 deferred
  to tb==3, and the sigmoid/num/den epilogue at chunk boundaries
  overlapping the next chunk's first transposes.
- The last chunk splits its final num/den matmuls and epilogue/f2/store
  by s-halves to shorten the end-of-kernel tail (which runs
  HAM-throttled at half PE clock).

Engine assignment: casting DMAs can only issue from gpsimd, so gpsimd
hosts the load descriptor-gen, emitted in per-group/per-chunk batches
~1.5 segments ahead of consumption and interleaved with gpsimd's only
compute (half of the expw8 fp8 casts, SBUF->SBUF since GPSIMD has no
PSUM port). Store descriptor-gen lives on DVE, directly after the
out-tile copies it depends on, so it never blocks. ACT is near-saturated
by the exp()s (the only exp-capable engine) and takes nothing else
heavy; all other PSUM reads stay on DVE.

Precision split (inherited from v1, hard-won): num is a SIGNED
accumulation, so every operand on the num path (x_kv, wk/wv, exp(K)*V,
exp(bias)) stays bf16. den is all-positive and Q only feeds a sigmoid
gate; both run fp8e4 MatmulPerfMode.DoubleRow (halved PE rows). The
output is stored bf16 (host casts back to f32).
"""

import os
import numpy as np
from contextlib import ExitStack

import concourse.bass as bass
import concourse.tile as tile
from concourse import bacc, mybir
from concourse.bass_utils import run_bass_kernel_spmd
from concourse.masks import make_identity

F32 = mybir.dt.float32
BF16 = mybir.dt.bfloat16
FP8 = mybir.dt.float8e4

S = 2048   # n_q
T = 2048   # n_kv
D = 1024   # d_q == d_kv
H = 256    # hidden
G = 256    # output dim
P = 128    # partitions
SCH = 512  # s-chunk for the C phase (one PSUM bank of fp32)
NSB = SCH // P       # 4 row-blocks per chunk
NCH = S // SCH       # 4 chunks
NT = T // P          # 16 t row-blocks
NG = NT // 2         # 8 groups of 2 t-blocks
ND = D // P          # 8 d tiles
NDD = ND // 2        # 4 d-tile pairs (DoubleRow)
NHB = H // P         # 2 h blocks
TQ = T // 4          # bias quarter length along t

DR = mybir.MatmulPerfMode.DoubleRow
AFT = mybir.ActivationFunctionType


def _build(use_wq_b, use_wk_b, use_wv_b, use_f2_b):
    """Build the per-core Bass graph. Returns the compiled Bacc."""
    nc = bacc.Bacc(
        "TRN2",
        target_bir_lowering=False,
        debug=False,
        enable_asserts=False,
        num_devices=8,
    )

    x_q = nc.declare_dram_parameter("x_q", [S, D], F32, isOutput=False)
    x_kv = nc.declare_dram_parameter("x_kv", [T, D], F32, isOutput=False)
    bias = nc.declare_dram_parameter("bias", [S, T], F32, isOutput=False)
    wq_w = nc.declare_dram_parameter("wq_w", [D, H], F32, isOutput=False)
    wk_w = nc.declare_dram_parameter("wk_w", [D, H], F32, isOutput=False)
    wv_w = nc.declare_dram_parameter("wv_w", [D, H], F32, isOutput=False)
    f2_w = nc.declare_dram_parameter("f2_w", [H, G], F32, isOutput=False)
    wq_b = nc.declare_dram_parameter("wq_b", [1, H], F32, isOutput=False) if use_wq_b else None
    wk_b = nc.declare_dram_parameter("wk_b", [1, H], F32, isOutput=False) if use_wk_b else None
    wv_b = nc.declare_dram_parameter("wv_b", [1, H], F32, isOutput=False) if use_wv_b else None
    f2_b = nc.declare_dram_parameter("f2_b", [1, G], F32, isOutput=False) if use_f2_b else None
    out = nc.declare_dram_parameter("out", [S, G], BF16, isOutput=True)

    # DRAM views with the chunk/block structure exposed.
    xq_v = x_q[:].rearrange("(c sb p) d -> c p sb d", p=P, sb=NSB)     # [NCH,P,NSB,D]
    xkv_v = x_kv[:].rearrange("(g j p) d -> g p j d", p=P, j=2)        # [NG,P,2,D]
    bias_v = bias[:].rearrange("(c sb p) (q t) -> c q p sb t", p=P, sb=NSB, q=4)
    out_v = out[:].rearrange("(c sb p) g -> c p sb g", p=P, sb=NSB)
    wk_v = wk_w[:].rearrange("(dt p) h -> p dt h", p=P)
    wv_v = wv_w[:].rearrange("(dt p) h -> p dt h", p=P)

    with tile.TileContext(nc) as tc, ExitStack() as ctx:
        consts = ctx.enter_context(tc.tile_pool(name="consts", bufs=1))

        ident = consts.tile([P, P], BF16)

        # Staging pools sized so the SWDGE queue never waits on a slot.
        xkv_nat = ctx.enter_context(tc.tile_pool(name="xkv_nat", bufs=2 * NG))
        xq_nat = ctx.enter_context(tc.tile_pool(name="xq_nat", bufs=NCH))
        bias_nat = ctx.enter_context(tc.tile_pool(name="bias_nat", bufs=12))
        xT_sb = ctx.enter_context(tc.tile_pool(name="xT_sb", bufs=8))
        xqT2_sb = ctx.enter_context(tc.tile_pool(name="xqT2_sb", bufs=8))
        expw_sb = ctx.enter_context(tc.tile_pool(name="expw_sb", bufs=6))
        expw8_sb = ctx.enter_context(tc.tile_pool(name="expw8_sb", bufs=4))
        epi = ctx.enter_context(tc.tile_pool(name="epi", bufs=3))
        out_sb_pool = ctx.enter_context(tc.tile_pool(name="out_sb", bufs=2))

        wkv_sb = consts.tile([P, ND, 2 * H], BF16, tag="w_wkv", name="wkv_sb")
        wq_bf = consts.tile([P, ND, H], BF16, tag="w_wq", name="wq_bf")
        wq_sb = consts.tile([P, ND, H], FP8, tag="w_wq8", name="wq_sb")
        f2_sb = consts.tile([P, NHB, G], BF16, tag="w_f2")

        # ---- load batches: emitted progressively on the gpsimd FIFO so
        # descriptor-gen stays ~1.5 segments ahead of consumption without a
        # long serial prefix (gpsimd also runs half the expw8 casts).
        xkv_tiles = {}
        bias_tiles = {}
        xq_tiles = {}

        def load_xkv(g):
            for j_ in range(2):
                t_ = xkv_nat.tile([P, D], BF16, tag="xkv_nat", name="xkv_nat")
                nc.gpsimd.dma_start(t_[:], xkv_v[g][:, j_, :])
                xkv_tiles[(g, j_)] = t_

        def load_bias_q(c, q):
            t_ = bias_nat.tile([P, NSB, TQ], BF16, tag="bias_nat", name="bias_nat")
            nc.gpsimd.dma_start(t_[:], bias_v[c][q])
            bias_tiles[(c, q)] = t_

        def load_xq(c):
            t_ = xq_nat.tile([P, NSB, D], BF16, tag="xq_nat", name="xq_nat")
            nc.gpsimd.dma_start(t_[:], xq_v[c])
            xq_tiles[c] = t_

        def load_wkv(half):
            dsl = slice(half * 4, half * 4 + 4)
            nc.gpsimd.dma_start(wkv_sb[:, dsl, 0:H], wk_v[:, dsl, :])
            nc.gpsimd.dma_start(wkv_sb[:, dsl, H : 2 * H], wv_v[:, dsl, :])

        def load_wq():
            nc.gpsimd.dma_start(wq_bf[:], wq_w[:].rearrange("(dt p) h -> p dt h", p=P))

        def load_f2():
            nc.gpsimd.dma_start(f2_sb[:], f2_w[:].rearrange("(ht p) g -> p ht g", p=P))

        # Upfront: just enough for group 0 + the first c0 quarter.
        load_xkv(0)
        make_identity(nc, ident[:])
        load_wkv(0)
        load_wkv(1)
        load_xkv(1)
        load_bias_q(0, 0)

        # Remaining loads, all upfront in consumption order: the gpsimd FIFO
        # carries ONLY descriptor-gen (and phase-2 Pool casts emitted after
        # every load), so the DMA stream can never be blocked by compute.
        load_xkv(2)
        load_xkv(3)
        load_bias_q(0, 1)
        load_xkv(4)
        load_xkv(5)
        load_bias_q(0, 2)
        load_wq()
        load_xkv(6)
        load_xq(0)
        load_xkv(7)
        load_bias_q(0, 3)
        load_bias_q(1, 0)
        load_xq(1)
        load_bias_q(1, 1)
        load_f2()
        load_bias_q(1, 2)
        load_bias_q(1, 3)
        load_xq(2)
        load_bias_q(2, 0)
        load_bias_q(2, 1)
        load_bias_q(2, 2)
        load_xq(3)
        load_bias_q(2, 3)
        load_bias_q(3, 0)
        load_bias_q(3, 1)
        load_bias_q(3, 2)
        load_bias_q(3, 3)

        bias_vecs = {}
        ones_row = None
        if any(b is not None for b in (wq_b, wk_b, wv_b, f2_b)):
            ones_row = consts.tile([1, SCH], BF16)
            nc.gpsimd.memset(ones_row[:], 1.0)
            for name, b in (("wq", wq_b), ("f2", f2_b)):
                if b is not None:
                    bt = consts.tile([1, H], BF16, tag=f"b_{name}", name="bt")
                    nc.gpsimd.dma_start(bt[:], b[:])
                    bias_vecs[name] = bt
            if wk_b is not None or wv_b is not None:
                bkv = consts.tile([1, 2 * H], BF16, tag="b_kv", name="bkv")
                nc.gpsimd.memset(bkv[:], 0.0)
                if wk_b is not None:
                    nc.gpsimd.dma_start(bkv[:, 0:H], wk_b[:])
                if wv_b is not None:
                    nc.gpsimd.dma_start(bkv[:, H : 2 * H], wv_b[:])
                bias_vecs["kv"] = bkv

        # Long-lived activations.
        enq_pool = ctx.enter_context(tc.tile_pool(name="enq", bufs=NHB))
        enq = [enq_pool.tile([P, S], BF16, tag="enq", name="enq") for _ in range(NHB)]
        ek_pool = ctx.enter_context(tc.tile_pool(name="expk", bufs=NG))
        ek8_pool = ctx.enter_context(tc.tile_pool(name="expk8", bufs=NG))
        ekv_pool = ctx.enter_context(tc.tile_pool(name="ekv", bufs=NG))
        expk = [ek_pool.tile([P, 2 * H], BF16, tag="expk", name="expk") for _ in range(NG)]
        expk8 = [ek8_pool.tile([P, 2, H], FP8, tag="expk8", name="expk8") for _ in range(NG)]
        ekv = [ekv_pool.tile([P, 2 * H], BF16, tag="ekv", name="ekv") for _ in range(NG)]

        # num/den accumulators span phase 1 (chunk 0) and phase 2: own pool.
        psum_nd = ctx.enter_context(tc.tile_pool(name="psum_nd", bufs=4, space="PSUM"))

        # ---------------- shared helpers ----------------
        def kv_exps(g, ps_kv):
            for j in range(2):
                nc.scalar.activation(
                    expk[g][:, j * H : (j + 1) * H], ps_kv[j][:, 0:H], AFT.Exp
                )
                nc.vector.tensor_mul(
                    ekv[g][:, j * H : (j + 1) * H],
                    expk[g][:, j * H : (j + 1) * H],
                    ps_kv[j][:, H : 2 * H],
                )
                # fp8 copy: DVE for early groups (ACT drains the K/V ring
                # faster); ACT's psum-exp for late groups (DVE is the
                # bottleneck at the phase-1 tail).
                if g >= 6:
                    nc.scalar.activation(expk8[g][:, j, :], ps_kv[j][:, 0:H], AFT.Exp)
                else:
                    nc.vector.tensor_copy(expk8[g][:, j, :], expk[g][:, j * H : (j + 1) * H])

        # C-phase state for the current chunk.
        cst = {}

        def c_open(c):
            cst["c"] = c
            cst["num"] = [psum_nd.tile([P, SCH], F32, tag="nd", name="ps_num") for _ in range(NHB)]
            cst["den"] = [psum_nd.tile([P, SCH], F32, tag="nd", name="ps_den") for _ in range(NHB)]
            cst["expw"] = {}
            cst["expw8"] = {}

        def c_tr(tb, tr_pool, tr_tag="tr"):
            c = cst["c"]
            nath = bias_tiles[(c, tb // 4)]
            tloc = (tb % 4) * P
            ps = tr_pool.tile([P, SCH], F32, tag=tr_tag, name="ps_trc")
            for sb in range(NSB):
                nc.tensor.matmul(
                    ps[:, sb * P : (sb + 1) * P],
                    nath[:, sb, tloc : tloc + P],
                    ident[:],
                )
            sbuf = expw_sb.tile([P, SCH], BF16, tag="expw", name="expw")
            nc.scalar.activation(sbuf[:], ps[:], AFT.Exp)
            cst["expw"][tb] = sbuf
            # fp8 copy for the den DoubleRow matmul. SBUF->SBUF, so mid-
            # chunk phase-2 copies can run on Pool (idle once desc-gen is
            # done; ~3x slower than DVE, so only where its latency hides);
            # the rest on DVE.
            g = tb // 2
            if tb % 2 == 0:
                cst["expw8"][g] = expw8_sb.tile([P, 2, SCH], FP8, tag="expw8", name="expw8")
            eng = nc.gpsimd if (cst["c"] > 0 and tb % 2 == 1 and tb <= 7) else nc.vector
            eng.tensor_copy(cst["expw8"][g][:, tb % 2, :], sbuf[:])

        def c_num(tb, first=None, last=None):
            first = (tb == 0) if first is None else first
            last = (tb == NT - 1) if last is None else last
            rhs = cst["expw"][tb][:]
            for hb in range(NHB):
                lo = (tb % 2) * H + hb * P
                nc.tensor.matmul(
                    cst["num"][hb][:],
                    ekv[tb // 2][:, lo : lo + P],
                    rhs,
                    start=first,
                    stop=last,
                )

        def c_num_half(tb, h):
            rhs = cst["expw"][tb][:, h * 2 * P : (h + 1) * 2 * P]
            for hb in range(NHB):
                lo = (tb % 2) * H + hb * P
                nc.tensor.matmul(
                    cst["num"][hb][:, h * 2 * P : (h + 1) * 2 * P],
                    ekv[tb // 2][:, lo : lo + P],
                    rhs,
                    start=False,
                    stop=True,
                )

        def c_den(g, first=None, last=None):
            first = (g == 0) if first is None else first
            last = (g == NG - 1) if last is None else last
            for hb in range(NHB):
                nc.tensor.matmul(
                    cst["den"][hb][:],
                    expk8[g][:, :, hb * P : (hb + 1) * P],
                    cst["expw8"][g][:],
                    start=first,
                    stop=last,
                    perf_mode=DR,
                )

        def c_den_half(g, h):
            for hb in range(NHB):
                nc.tensor.matmul(
                    cst["den"][hb][:, h * 2 * P : (h + 1) * 2 * P],
                    expk8[g][:, :, hb * P : (hb + 1) * P],
                    cst["expw8"][g][:, :, h * 2 * P : (h + 1) * 2 * P],
                    start=False,
                    stop=True,
                    perf_mode=DR,
                )

        # B-phase helpers (x_q transpose + Q DR matmul for chunk c).
        bst = {}

        def b_open(c):
            bst["c"] = c
            bst["xqT2"] = {}

        def b_tr(d, tr_pool, tr_tag="tr"):
            nat = xq_tiles[bst["c"]]
            ps = tr_pool.tile([P, SCH], F32, tag=tr_tag, name="ps_trq")
            for sb in range(NSB):
                nc.tensor.matmul(
                    ps[:, sb * P : (sb + 1) * P],
                    nat[:, sb, d * P : (d + 1) * P],
                    ident[:],
                )
            if d % 2 == 0:
                bst["xqT2"][d // 2] = xqT2_sb.tile(
                    [P, 2, SCH], FP8, tag="xqT2", name="xqT2"
                )
            if d < 2:
                nc.scalar.copy(bst["xqT2"][d // 2][:, d % 2, :], ps[:])
            else:
                nc.vector.tensor_copy(bst["xqT2"][d // 2][:, d % 2, :], ps[:])

        def b_mm(dd, q_pool, q_tag):
            if dd == 0:
                bst["ps_q"] = [
                    q_pool.tile([P, SCH], F32, tag=q_tag, name="ps_q") for _ in range(NHB)
                ]
            for hb in range(NHB):
                nc.tensor.matmul(
                    bst["ps_q"][hb][:],
                    wq_sb[:, 2 * dd : 2 * dd + 2, hb * P : (hb + 1) * P],
                    bst["xqT2"][dd][:],
                    start=(dd == 0),
                    stop=(dd == NDD - 1 and "wq" not in bias_vecs),
                    perf_mode=DR,
                )

        def b_close():
            c = bst["c"]
            for hb in range(NHB):
                if "wq" in bias_vecs:
                    nc.tensor.matmul(
                        bst["ps_q"][hb][:],
                        bias_vecs["wq"][:, hb * P : (hb + 1) * P],
                        ones_row[:],
                        start=False,
                        stop=True,
                    )
                nc.scalar.activation(
                    enq[hb][:, c * SCH : (c + 1) * SCH],
                    bst["ps_q"][hb][:],
                    AFT.Sigmoid,
                )

        def epilogue(c):
            # Yt^T = num^T * (recip(den^T) * sig(Q)^T). The recip/mul chain
            # depends only on den and the sigmoid gate, so when the dens
            # finish before the last num matmuls, only the final muls trail
            # the num-stop.
            rec = []
            for hb in range(NHB):
                r = epi.tile([P, SCH], F32, tag="rec", name="rec", bufs=2)
                nc.vector.reciprocal_approx_fast(r[:], cst["den"][hb][:])
                rec.append(r)
            m = []
            for hb in range(NHB):
                mm_ = epi.tile([P, SCH], F32, tag="d2", name="m", bufs=2)
                nc.vector.tensor_mul(
                    mm_[:], rec[hb][:], enq[hb][:, c * SCH : (c + 1) * SCH]
                )
                m.append(mm_)
            ytT = []
            for hb in range(NHB):
                yt = epi.tile([P, SCH], BF16, tag="yt", name="yt", bufs=4)
                nc.vector.tensor_mul(yt[:], m[hb][:], cst["num"][hb][:])
                ytT.append(yt)
            return ytT

        def emit_f2(ytT, c, f_pool, f_tag):
            # Deferred f2 projection + store for chunks 0-2.
            out_sb = out_sb_pool.tile([P, NSB, G], BF16, tag="out_sb", name="out_sb")
            for sb in range(NSB):
                ps_f = f_pool.tile([P, SCH], F32, tag=f_tag, name="ps_f")
                for hb in range(NHB):
                    nc.tensor.matmul(
                        ps_f[:, 0:G],
                        ytT[hb][:, sb * P : (sb + 1) * P],
                        f2_sb[:, hb, :],
                        start=(hb == 0),
                        stop=(hb == NHB - 1 and "f2" not in bias_vecs),
                    )
                if "f2" in bias_vecs:
                    nc.tensor.matmul(
                        ps_f[:, 0:G],
                        ones_row[:, 0:P],
                        bias_vecs["f2"][:],
                        start=False,
                        stop=True,
                    )
                nc.vector.tensor_copy(out_sb[:, sb, :], ps_f[:, 0:G])
            # Stores are non-casting -> hardware DGE on SP (own queue).
            nc.sync.dma_start(out_v[c], out_sb[:])

        # ---------------- Phase 1: A + C0 (+ B0 transposes) ----------------
        # One psum layout for the whole kernel (no pool close/open barrier at
        # the phase transition): trc holds every transpose, fq holds the K/V
        # accumulators, B ps_q, and f2 blocks -- all [P,512] tiles whose
        # lifetimes never overlap across uses.
        psum_trc = ctx.enter_context(tc.tile_pool(name="psum_trc", bufs=2, space="PSUM"))
        psum_fq = ctx.enter_context(tc.tile_pool(name="psum_fq", bufs=2, space="PSUM"))

        c_open(0)
        b_open(0)

        # Warm-up filler: ~4us of ident matmuls into a throwaway psum tile.
        # The PE would idle here waiting for the first x_kv tiles; staying
        # busy keeps the HAM clock gate from parking the PE at half speed
        # for the first real groups. The tile has no readers, so the ring
        # slot recycles on WAR against the PE itself.
        warm = psum_trc.tile([P, SCH], F32, tag="trc", name="ps_warm")
        for _ in range(20):
            for k4 in range(SCH // P):
                nc.tensor.matmul(warm[:, k4 * P : (k4 + 1) * P], ident[:], ident[:])

        for g in range(NG):
            ps_kv = [psum_fq.tile([P, 2 * H], F32, tag="fq", name="ps_kv") for _ in range(2)]
            xkvT = {}

            def a_tr2(dp, g=g, xkvT=xkvT):
                # Transpose d-pair (2d, 2d+1) x (j=0,1) into one psum bank;
                # a single [P,512] DVE cast amortizes psum-access + semaphore
                # overhead (DVE is phase-1's co-bottleneck).
                ps = psum_trc.tile([P, SCH], F32, tag="trc", name="ps_tr")
                for k in range(2):
                    for j in range(2):
                        nc.tensor.matmul(
                            ps[:, (2 * k + j) * P : (2 * k + j + 1) * P],
                            xkv_tiles[(g, j)][:, (2 * dp + k) * P : (2 * dp + k + 1) * P],
                            ident[:],
                        )
                t_ = xT_sb.tile([P, 4 * P], BF16, tag="xkvT", name="xkvT")
                nc.vector.tensor_copy(t_[:], ps[:])
                xkvT[dp] = t_

            def a_mm(d, ps_kv=ps_kv, xkvT=xkvT):
                t_ = xkvT[d // 2]
                for j in range(2):
                    nc.tensor.matmul(
                        ps_kv[j][:],
                        t_[:, ((d % 2) * 2 + j) * P : ((d % 2) * 2 + j + 1) * P],
                        wkv_sb[:, d, :],
                        start=(d == 0),
                        stop=(d == ND - 1 and "kv" not in bias_vecs),
                    )

            for d in range(ND):
                if d % 2 == 0:
                    a_tr2(d // 2)
                # Lagged chunk-0 C work: group g handles tb = 2(g-2),
                # 2(g-2)+1 transposes, tb = 2(g-3)(+1) num, and den(g-3).
                if d == 1 and g >= 2:
                    c_tr(2 * (g - 2), psum_trc, "trc")
                if d == 2 and g >= 3:
                    c_num(2 * (g - 3))
                if d >= 3:
                    a_mm(d - 3)
                if d == 4 and g >= 2:
                    c_tr(2 * (g - 2) + 1, psum_trc, "trc")
                if d == 5 and g >= 3:
                    c_num(2 * (g - 3) + 1)
                if d == 6 and g >= 3:
                    c_den(g - 3)
                # B0 transposes ride along with group 7.
                if g == 7 and d % 2 == 1:
                    b_tr((d - 1) // 2, psum_trc, "trc")
            a_mm(ND - 3)
            a_mm(ND - 2)
            a_mm(ND - 1)
            if "kv" in bias_vecs:
                for j in range(2):
                    nc.tensor.matmul(
                        ps_kv[j][:],
                        ones_row[:, 0:P],
                        bias_vecs["kv"][:],
                        start=False,
                        stop=True,
                    )
            kv_exps(g, ps_kv)
            # wq bf16 -> fp8 conversion on ACT, spread over groups 5-7 (wq
            # lands mid-phase-1; each piece is one DR stationary pair).
            if g >= 5:
                dd = g - 5
                nc.scalar.copy(
                    wq_sb[:, 2 * dd : 2 * dd + 2, :], wq_bf[:, 2 * dd : 2 * dd + 2, :]
                )

        # Phase-1 tail: finish chunk 0 and B0 on the freed K/V banks. By now
        # the load queue is nearly drained, so this also keeps the PE busy.
        c_tr(12, psum_trc, "trc")
        c_num(10)
        b_tr(4, psum_trc, "trc")
        c_tr(13, psum_trc, "trc")
        c_num(11)
        c_den(5)
        nc.scalar.copy(wq_sb[:, 6:8, :], wq_bf[:, 6:8, :])
        b_tr(5, psum_trc, "trc")
        c_tr(14, psum_trc, "trc")
        c_num(12)
        b_tr(6, psum_trc, "trc")
        c_tr(15, psum_trc, "trc")
        c_num(13)
        b_tr(7, psum_trc, "trc")
        c_den(6)
        b_mm(0, psum_fq, "fq")
        c_den(7)
        b_mm(1, psum_fq, "fq")
        c_num(14)
        b_mm(2, psum_fq, "fq")
        c_num(15)
        b_mm(3, psum_fq, "fq")
        b_close()
        ytT0 = epilogue(0)
        pending_f2 = [(ytT0, 0)]

        # ---------------- Phase 2: chunks 1-3 ----------------
        if True:
            for c in range(1, NCH):
                c_open(c)
                b_open(c)
                last = c == NCH - 1
                # tb visit order: 10,11 go last so every den group (and
                # den's DVE casts) completes before the final num matmuls --
                # the epilogue's recip/mul chain then overlaps nums 10/11.
                seq = list(range(10)) + [12, 13, 14, 15, 10, 11]
                den_at = {4: 0, 6: 1, 8: 2, 10: 3, 12: 4, 13: 6, 15: 7}
                for pos in range(NT):
                    tb = seq[pos]
                    c_tr(tb, psum_trc, "trc")
                    if pos == 6 and pending_f2[0] is not None:
                        emit_f2(*pending_f2[0], psum_fq, "fq")
                        pending_f2[0] = None
                    if pos >= 2:
                        tn = seq[pos - 2]
                        c_num(tn, first=(tn == 0), last=False)
                    if 3 <= pos <= 10:
                        b_tr(pos - 3, psum_trc, "trc")
                    if pos in (7, 9, 11, 13):
                        b_mm((pos - 7) // 2, psum_fq, "fq")
                    if pos in den_at:
                        g_ = den_at[pos]
                        c_den(g_, first=(g_ == 0), last=False)
                    if pos == 15:
                        # After c_tr(11)'s exp is on the ACT FIFO: the two
                        # sigmoids must not delay the trailing nums' exps.
                        b_close()
                # dens finish (den 5 last), then the trailing nums.
                c_den(5, first=False, last=True)
                c_num(10, first=False, last=False)
                c_num(11, first=False, last=True)
                ytT = epilogue(c)
                if not last:
                    pending_f2[0] = (ytT, c)
                else:
                    out_sb = out_sb_pool.tile([P, NSB, G], BF16, tag="out_sb", name="out_sb")
                    for sb in range(NSB):
                        ps_f = psum_fq.tile([P, SCH], F32, tag="fq", name="ps_fh")
                        for hb in range(NHB):
                            nc.tensor.matmul(
                                ps_f[:, 0:G],
                                ytT[hb][:, sb * P : (sb + 1) * P],
                                f2_sb[:, hb, :],
                                start=(hb == 0),
                                stop=(hb == NHB - 1 and "f2" not in bias_vecs),
                            )
                        if "f2" in bias_vecs:
                            nc.tensor.matmul(
                                ps_f[:, 0:G],
                                ones_row[:, 0:P],
                                bias_vecs["f2"][:],
                                start=False,
                                stop=True,
                            )
                        nc.vector.tensor_copy(out_sb[:, sb, :], ps_f[:, 0:G])
                        nc.sync.dma_start(out_v[c][:, sb, :], out_sb[:, sb, :])

    nc.compile()
    return nc


_CACHE = {}


def _get_nc(use_wq_b, use_wk_b, use_wv_b, use_f2_b):
    key = (use_wq_b, use_wk_b, use_wv_b, use_f2_b)
    if key not in _CACHE:
        _CACHE[key] = _build(*key)
    return _CACHE[key]


def kernel(x_q, x_kv, bias, wq_w, wq_b, wk_w, wk_b, wv_w, wv_b, f2_w, f2_b,
           _trace=False, _trace_kwargs=None):
    x_q = np.ascontiguousarray(np.asarray(x_q, dtype=np.float32))
    x_kv = np.ascontiguousarray(np.asarray(x_kv, dtype=np.float32))
    bias = np.ascontiguousarray(np.asarray(bias, dtype=np.float32))
    wq_w = np.ascontiguousarray(np.asarray(wq_w, dtype=np.float32))
    wk_w = np.ascontiguousarray(np.asarray(wk_w, dtype=np.float32))
    wv_w = np.ascontiguousarray(np.asarray(wv_w, dtype=np.float32))
    f2_w = np.ascontiguousarray(np.asarray(f2_w, dtype=np.float32))
    wq_b = np.asarray(wq_b, dtype=np.float32)
    wk_b = np.asarray(wk_b, dtype=np.float32)
    wv_b = np.asarray(wv_b, dtype=np.float32)
    f2_b = np.asarray(f2_b, dtype=np.float32)

    use_b = tuple(bool(np.any(b)) for b in (wq_b, wk_b, wv_b, f2_b))
    nc = _get_nc(*use_b)

    n_cores = 8
    in_maps = []
    for i in range(n_cores):
        m = {
            "x_q": x_q[i],
            "x_kv": x_kv[i],
            "bias": bias[i],
            "wq_w": wq_w,
            "wk_w": wk_w,
            "wv_w": wv_w,
            "f2_w": f2_w,
        }
        if use_b[0]:
            m["wq_b"] = wq_b.reshape(1, H)
        if use_b[1]:
            m["wk_b"] = wk_b.reshape(1, H)
        if use_b[2]:
            m["wv_b"] = wv_b.reshape(1, H)
        if use_b[3]:
            m["f2_b"] = f2_b.reshape(1, G)
        in_maps.append(m)

    if not _trace:
        # The NTFF trace hook is unavailable outside the dev harness; make
        # sure a stray BASS_TRACE env var cannot route us onto that path.
        os.environ["BASS_NEVER_TRACE"] = "1"
    else:
        os.environ.pop("BASS_NEVER_TRACE", None)
    res = run_bass_kernel_spmd(
        nc, in_maps, list(range(n_cores)), trace=_trace, **(_trace_kwargs or {})
    )
    out = np.stack(
        [np.asarray(res.results[i]["out"]).astype(np.float32) for i in range(n_cores)],
        axis=0,
    )
    if _trace:
        return out, res
    return out
